# revision 5
# baseline (speedup 1.0000x reference)
"""Causal self-attention (B=4, S=2048, E=1024, H=16) on 8 TRN2 NeuronCores.

Sharding: core = (batch b, head-group g): b = core // 2, g = core % 2.
Each core handles one batch and 8 of the 16 heads (Megatron-style column
parallel QKV + row-parallel out-proj); the two half-projections per batch
are summed on the host.

All weights/activations are pre-transposed and cast to bf16 on the host so
the on-chip program is pure matmul + softmax:
  qT/kT [j=h*64+d, s] = WT.T @ xT        (heads on partitions)
  v     [s, j]        = xT.T @ WvT       (natural layout, + ones column)
  scT   [kj, qi]      = kT_h.T' @ qT_h   (K=64, 2 heads row-packed in PE)
  e = exp(0.125*scT)                     (ScalarE from PSUM)
  e *= causal                            (GpSimd affine_select, diag tiles)
  pv    [65, qi]      = [v_h | 1].T @ e  (accumulated over kj; row 64 = rowsum)
  o     = pv[0:64] / pv[64]              (DVE fast recip + GpSimd part-bcast)
  outT  [e, s]        = WpT.T @ o_cat    (partial, bf16; host sums the groups)

Scheduling: the attention inner loop is ScalarE(exp)-paced, so all other
matmul streams (QKV projections, V, out-proj) are chopped into ~0.4-0.9us
"filler quanta" and woven into the attention emission at TILE granularity
by a virtual-clock greedy weaver: whenever the PE would have to wait for an
exp to finish, it runs filler instead.  The kernel opens with a kt-major
Q-projection bootstrap that chases the x DMA stream chunk by chunk, and the
ScalarE instruction stream carries nothing but exps (DMA configs live on
sync/gpsimd/vector).  Output is written bf16 (host accumulates in f32).
"""

import sys

for _p in ("/opt/trn_rl_repo", "/root/.axon_site/_ro/trn_rl_repo"):
    if _p not in sys.path:
        sys.path.append(_p)

from contextlib import ExitStack

import numpy as np
import ml_dtypes

import concourse.bass as bass
import concourse.tile as tile
import concourse.mybir as mybir
from concourse import bacc
from concourse.bass_utils import run_bass_kernel_spmd

BF16 = mybir.dt.bfloat16
F32 = mybir.dt.float32
NP_BF16 = ml_dtypes.bfloat16

B, S, E, H = 4, 2048, 1024, 16
D = E // H            # 64
HL = H // 2           # 8 heads per core
JC = HL * D           # 512 local head-concat width
P = 128
NKT = S // P          # 16 key tiles
NQT = S // 512        # 4 query tiles of 512
EKT = E // P          # 8 contraction tiles for QKV projections
CT = JC // P          # 4 contraction tiles for the output projection
SCALE = 1.0 / np.sqrt(np.float32(D))  # 0.125

# virtual-clock estimates (ns)
CYC = 1.0 / 2.4
SEM = 100.0


def build_program(apply_key_mask: bool):
    nc = bacc.Bacc("TRN2", target_bir_lowering=False, debug=False, num_devices=8)

    xT = nc.dram_tensor("xT", [E, S], BF16, kind="ExternalInput").ap()
    wqT = nc.dram_tensor("wqT", [E, JC], BF16, kind="ExternalInput").ap()
    wkT = nc.dram_tensor("wkT", [E, JC], BF16, kind="ExternalInput").ap()
    wvT = nc.dram_tensor("wvT", [E, JC], BF16, kind="ExternalInput").ap()
    wpT = nc.dram_tensor("wpT", [JC, E], BF16, kind="ExternalInput").ap()
    if apply_key_mask:
        kmaskT = nc.dram_tensor("kmaskT", [P, NKT], F32, kind="ExternalInput").ap()
    outp = nc.dram_tensor("outp", [E, S], BF16, kind="ExternalOutput").ap()

    xT_r = xT.rearrange("(kt p) s -> p kt s", p=P)
    wq_r = wqT.rearrange("(kt p) j -> p kt j", p=P)
    wk_r = wkT.rearrange("(kt p) j -> p kt j", p=P)
    wv_r = wvT.rearrange("(kt p) j -> p kt j", p=P)

    with tile.TileContext(nc) as tc:
        with ExitStack() as ctx:
            per = ctx.enter_context(tc.tile_pool(name="per", bufs=1))
            sc_ps = ctx.enter_context(
                tc.tile_pool(name="sc_ps", bufs=2, space="PSUM")
            )
            pv_ps = ctx.enter_context(
                tc.tile_pool(name="pv_ps", bufs=2, space="PSUM")
            )
            fill_ps = ctx.enter_context(
                tc.tile_pool(name="fill_ps", bufs=2, space="PSUM")
            )
            esb = ctx.enter_context(tc.tile_pool(name="esb", bufs=3))
            nrm = ctx.enter_context(tc.tile_pool(name="nrm", bufs=6))
            posb = ctx.enter_context(tc.tile_pool(name="posb", bufs=3))

            # ---- input DMA: x chunk-wise on sync/gpsimd (chaseable), weights
            # as few-config batched transfers on the vector queue; ScalarE
            # stays untouched so its stream is pure exp.
            wq_sb = per.tile([P, EKT, JC], BF16, tag="wq")
            wk_sb = per.tile([P, EKT, JC], BF16, tag="wk")
            wv_sb = per.tile([P, EKT, JC], BF16, tag="wv")
            xT_sb = per.tile([P, EKT, S], BF16, tag="xT")
            for kt in range(EKT):
                (nc.sync if kt % 2 == 0 else nc.gpsimd).dma_start(
                    xT_sb[:, kt], xT_r[:, kt]
                )
            for h in range(2):
                hs = slice(4 * h, 4 * h + 4)
                nc.scalar.dma_start(wq_sb[:, hs], wq_r[:, hs])
            for h in range(2):
                hs = slice(4 * h, 4 * h + 4)
                nc.scalar.dma_start(wk_sb[:, hs], wk_r[:, hs])
            for h in range(2):
                hs = slice(4 * h, 4 * h + 4)
                nc.scalar.dma_start(wv_sb[:, hs], wv_r[:, hs])
            wp_sb = per.tile([P, CT, E], BF16, tag="wp")
            nc.scalar.dma_start(wp_sb[:], wpT.rearrange("(ct p) e -> p ct e", p=P))
            if apply_key_mask:
                km_sb = per.tile([P, NKT], F32, tag="km")
                nc.sync.dma_start(km_sb[:], kmaskT[:])

            qT_sb = per.tile([P, CT, S], BF16, tag="qT")
            kT_sb = per.tile([P, CT, S], BF16, tag="kT")
            vaug_sb = per.tile([P, NKT, HL, D + 1], BF16, tag="vaug")
            o_sb = per.tile([P, CT, S], BF16, tag="o")

            nc.vector.memset(vaug_sb[:, :, :, D], 1.0)

            # dummy matmuls during the DMA-bound start: engage the HAM clock
            # gate before the first x chunks land so the bootstrap runs fast
            dum_a = per.tile([P, P], BF16, tag="dum_a")
            dum_b = per.tile([P, 256], BF16, tag="dum_b")
            nc.vector.memset(dum_a[:], 0.0)
            nc.vector.memset(dum_b[:], 0.0)
            dps = fill_ps.tile([P, 512], F32, tag="ps")
            for i in range(10):
                nc.tensor.matmul(
                    dps[:, 0:256], dum_a[:], dum_b[:],
                    start=(i == 0), stop=(i == 9),
                )

            # preload the exp table on ScalarE while DMAs stream in
            warm = nrm.tile([1, 16], F32, tag="warm")
            nc.vector.memset(warm[:], 0.0)
            warm2 = nrm.tile([1, 16], F32, tag="warm2")
            nc.scalar.activation(
                warm2[:], warm[:], mybir.ActivationFunctionType.Exp
            )

            # ================= emission helpers =================

            def emit_qk_unit(w_sb, dst, jt, st, ek_lo, ek_hi, ps_hold):
                """Half of one (w, jt, st) projection: accumulate ek chunks;
                close + copy out on the second half."""
                if ek_lo == 0:
                    ps_hold["t"] = fill_ps.tile([P, 512], F32, tag="ps", name="fillps")
                ps = ps_hold["t"]
                for ek in range(ek_lo, ek_hi):
                    nc.tensor.matmul(
                        ps[:],
                        w_sb[:, ek, jt * P : (jt + 1) * P],
                        xT_sb[:, ek, st * 512 : (st + 1) * 512],
                        start=(ek == 0),
                        stop=(ek == EKT - 1),
                    )
                if ek_hi == EKT:
                    nc.vector.tensor_copy(
                        dst[:, jt, st * 512 : (st + 1) * 512], ps[:]
                    )

            def emit_v_unit(s128, ek_lo, ek_hi, ps_hold):
                if ek_lo == 0:
                    ps_hold["t"] = fill_ps.tile([P, 512], F32, tag="ps", name="fillps")
                ps = ps_hold["t"]
                for ek in range(ek_lo, ek_hi):
                    nc.tensor.matmul(
                        ps[:],
                        xT_sb[:, ek, s128 * P : (s128 + 1) * P],
                        wv_sb[:, ek, :],
                        start=(ek == 0),
                        stop=(ek == EKT - 1),
                    )
                if ek_hi == EKT:
                    nc.vector.tensor_copy(
                        vaug_sb[:, s128, :, 0:D],
                        ps[:].rearrange("p (h d) -> p h d", d=D),
                    )
                    if apply_key_mask:
                        nc.vector.tensor_scalar_mul(
                            vaug_sb[:, s128], vaug_sb[:, s128],
                            km_sb[:, s128 : s128 + 1],
                        )

            def emit_proj_unit(st, et, ct_lo, ct_hi, ps_hold):
                if ct_lo == 0:
                    ps_hold["t"] = fill_ps.tile([P, 512], F32, tag="ps", name="fillps")
                ps = ps_hold["t"]
                for ct in range(ct_lo, ct_hi):
                    nc.tensor.matmul(
                        ps[:],
                        wp_sb[:, ct, et * P : (et + 1) * P],
                        o_sb[:, ct, st * 512 : (st + 1) * 512],
                        start=(ct == 0),
                        stop=(ct == CT - 1),
                    )
                if ct_hi == CT:
                    po = posb.tile([P, 512], BF16, tag="po")
                    nc.vector.tensor_copy(po[:], ps[:])
                    nc.sync.dma_start(
                        outp[et * P : (et + 1) * P, st * 512 : (st + 1) * 512],
                        po[:],
                    )

            # ================= the weaver =================

            class Weaver:
                def __init__(self):
                    self.items = []   # [gate, emit_fn, pe_ns, tag]
                    self.progress = 0
                    self.pe_t = 0.0
                    self.sc_t = 0.0

                def add(self, emit, ns, tag=None, gate=0):
                    self.items.append([gate, emit, ns, tag])

                def pe(self, ns):
                    self.pe_t += ns

                def drain_until(self, tag):
                    idx = None
                    for i, it in enumerate(self.items):
                        if it[3] == tag:
                            idx = i
                            break
                    if idx is None:
                        return
                    for it in self.items[: idx + 1]:
                        it[1]()
                        self.pe_t += it[2]
                    del self.items[: idx + 1]

                def has(self, tag):
                    return any(it[3] == tag for it in self.items)

                def fill(self, target):
                    while self.pe_t < target:
                        pick = None
                        for it in self.items:
                            if it[0] <= self.progress:
                                pick = it
                                break
                        if pick is None:
                            return
                        self.items.remove(pick)
                        pick[1]()
                        self.pe_t += pick[2]

                def drain_all(self):
                    # emit every remaining item (all gates satisfied by the end)
                    while self.items:
                        it = self.items.pop(0)
                        it[1]()
                        self.pe_t += it[2]

            W = Weaver()

            # ---- bootstrap: q-projection jt=0, all st, kt-major (chases the
            # x chunk stream); then kT jt=0 st=0 so attention can start.
            # Two sc tiles each hold two q accumulation groups side by side.
            qA = sc_ps.tile([P, 1024], F32, tag="sc")
            qB = sc_ps.tile([P, 1024], F32, tag="sc")
            q_groups = [
                qA[:, 0:512], qA[:, 512:1024], qB[:, 0:512], qB[:, 512:1024]
            ]
            for ek in range(EKT):
                for st in range(NQT):
                    nc.tensor.matmul(
                        q_groups[st],
                        wq_sb[:, ek, 0:P],
                        xT_sb[:, ek, st * 512 : (st + 1) * 512],
                        start=(ek == 0),
                        stop=(ek == EKT - 1),
                        skip_group_check=True,
                    )
            W.pe(EKT * NQT * 512 * CYC)
            # copy q st0 first so attention unblocks earliest
            for st in range(NQT):
                nc.vector.tensor_copy(
                    qT_sb[:, 0, st * 512 : (st + 1) * 512], q_groups[st]
                )

            k0_ps = pv_ps.tile([P, 512], F32, tag="pv")
            for ek in range(EKT):
                nc.tensor.matmul(
                    k0_ps[:],
                    wk_sb[:, ek, 0:P],
                    xT_sb[:, ek, 0:512],
                    start=(ek == 0),
                    stop=(ek == EKT - 1),
                    skip_group_check=True,
                )
            W.pe(EKT * 512 * CYC)
            nc.vector.tensor_copy(kT_sb[:, 0, 0:512], k0_ps[:])

            # ---- filler inventory
            def add_qk(jt, st, w_sb, dst, pre):
                hold = {}
                W.add(lambda h=hold: emit_qk_unit(w_sb, dst, jt, st, 0, 4, h),
                      4 * 512 * CYC, tag=None)
                W.add(lambda h=hold: emit_qk_unit(w_sb, dst, jt, st, 4, 8, h),
                      4 * 512 * CYC, tag=f"{pre}{jt}_{st}")

            def add_v(s128):
                hold = {}
                W.add(lambda h=hold: emit_v_unit(s128, 0, 4, h),
                      4 * 512 * CYC, tag=None)
                W.add(lambda h=hold: emit_v_unit(s128, 4, 8, h),
                      4 * 512 * CYC, tag=f"v{s128}")

            def add_proj(st, gate):
                for et in range(8):
                    hold = {}
                    W.add(lambda e=et, h=hold: emit_proj_unit(st, e, 0, 2, h),
                          2 * 512 * CYC, gate=gate)
                    W.add(lambda e=et, h=hold: emit_proj_unit(st, e, 2, 4, h),
                          2 * 512 * CYC, gate=gate)

            # remaining kT jt=0 tiles
            for st in range(1, NQT):
                hold = {}
                W.add(lambda s=st, h=hold: emit_qk_unit(wk_sb, kT_sb, 0, s, 0, 4, h),
                      4 * 512 * CYC)
                W.add(lambda s=st, h=hold: emit_qk_unit(wk_sb, kT_sb, 0, s, 4, 8, h),
                      4 * 512 * CYC, tag=f"k0_{st}")
            for s128 in range(4):
                add_v(s128)
            for a in range(1, 4):
                add_qk(a, 0, wq_sb, qT_sb, "q")
                add_qk(a, 0, wk_sb, kT_sb, "k")
            for s128 in range(4, 8):
                add_v(s128)
            for a in range(1, 4):
                add_qk(a, 1, wq_sb, qT_sb, "q")
                add_qk(a, 1, wk_sb, kT_sb, "k")
            add_proj(0, gate=4)
            for s128 in range(8, 12):
                add_v(s128)
            for a in range(1, 4):
                add_qk(a, 2, wq_sb, qT_sb, "q")
                add_qk(a, 2, wk_sb, kT_sb, "k")
            add_proj(1, gate=8)
            for s128 in range(12, 16):
                add_v(s128)
            for a in range(1, 4):
                add_qk(a, 3, wq_sb, qT_sb, "q")
                add_qk(a, 3, wk_sb, kT_sb, "k")
            add_proj(2, gate=12)
            add_proj(3, gate=16)

            # ---- attention blocks, tile-granular weave
            def emit_attn(qt, a):
                qs0 = qt * 512
                nkt = 4 * qt + 4
                pv = pv_ps.tile([P, 512], F32, tag="pv")
                pv2 = pv_ps.tile([P, 512], F32, tag="pv")

                def emit_sc(kt):
                    r = kt - 4 * qt
                    c0 = 128 * r if r > 0 else 0
                    ks = slice(kt * P, (kt + 1) * P)
                    qs = slice(qs0 + c0, qs0 + 512)
                    sc = sc_ps.tile([P, 1024], F32, tag="sc")
                    nc.tensor.matmul(
                        sc[:, c0:512],
                        kT_sb[0:D, a, ks],
                        qT_sb[0:D, a, qs],
                        start=True, stop=True,
                    )
                    nc.tensor.matmul(
                        sc[:, 512 + c0 : 1024],
                        kT_sb[D : 2 * D, a, ks],
                        qT_sb[D : 2 * D, a, qs],
                        start=True, stop=True,
                    )
                    W.pe(2 * (512 - c0) * CYC + 40)
                    return sc

                def emit_exp(sc, kt):
                    r = kt - 4 * qt
                    c0 = 128 * r if r > 0 else 0
                    e = esb.tile([P, 1024], BF16, tag="e")
                    e2 = e[:].rearrange("p (two c) -> p two c", two=2)
                    sc2 = sc[:].rearrange("p (two c) -> p two c", two=2)
                    nc.scalar.activation(
                        e2[:, :, c0:512], sc2[:, :, c0:512],
                        mybir.ActivationFunctionType.Exp,
                        scale=float(SCALE),
                    )
                    exp_ns = 2 * (512 - c0) * 0.8333 + 215
                    start = max(W.sc_t, W.pe_t + SEM)
                    W.sc_t = start + exp_ns
                    dep_end = W.sc_t
                    if r >= 0:
                        # causal mask on the first 128-col subblock: keep
                        # element iff q_local >= key_partition
                        nc.gpsimd.affine_select(
                            out=e2[:, :, c0 : c0 + 128],
                            in_=e2[:, :, c0 : c0 + 128],
                            compare_op=mybir.AluOpType.is_ge,
                            fill=0.0,
                            base=0,
                            pattern=[[0, 2], [1, 128]],
                            channel_multiplier=-1,
                        )
                        dep_end += 480 + SEM
                    return e, dep_end

                def emit_pv(e, kt, last):
                    r = kt - 4 * qt
                    c0 = 128 * r if r > 0 else 0
                    nc.tensor.matmul(
                        pv[0 : D + 1, c0:512],
                        vaug_sb[:, kt, 2 * a, :],
                        e[:, c0:512],
                        start=(kt == 0), stop=last,
                        skip_group_check=True,
                    )
                    nc.tensor.matmul(
                        pv2[0 : D + 1, c0:512],
                        vaug_sb[:, kt, 2 * a + 1, :],
                        e[:, 512 + c0 : 1024],
                        start=(kt == 0), stop=last,
                        skip_group_check=True,
                    )
                    W.pe(2 * (512 - c0) * CYC + 40)

                sc = emit_sc(0)
                for kt in range(nkt):
                    e, dep_end = emit_exp(sc, kt)
                    if kt + 1 < nkt:
                        sc = emit_sc(kt + 1)
                    if W.has(f"v{kt}"):
                        W.drain_until(f"v{kt}")
                    W.fill(dep_end + 50)
                    if W.pe_t < dep_end + 50:
                        W.pe_t = dep_end + 50
                    emit_pv(e, kt, last=(kt == nkt - 1))

                # normalize: o = pv[0:64] * (1 / pv[64])
                qsl = slice(qs0, qs0 + 512)
                for h_par, pvt in ((0, pv), (1, pv2)):
                    u = nrm.tile([D + 1, 512], F32, tag="unorm")
                    nc.vector.tensor_copy(u[:], pvt[0 : D + 1, :])
                    rsum = nrm.tile([1, 512], F32, tag="rsum")
                    nc.vector.tensor_copy(rsum[:], u[D : D + 1, :])
                    rec = nrm.tile([1, 512], F32, tag="rec")
                    nc.vector.reciprocal_approx_fast(rec[:], rsum[:])
                    bc = nrm.tile([D, 512], F32, tag="bc")
                    nc.gpsimd.partition_broadcast(bc[:], rec[:])
                    if h_par == 0:
                        nc.vector.tensor_mul(o_sb[0:D, a, qsl], u[0:D, :], bc[:])
                    else:
                        tmp = nrm.tile([D, 512], BF16, tag="tmp")
                        nc.vector.tensor_mul(tmp[:], u[0:D, :], bc[:])
                        nc.gpsimd.dma_start(o_sb[D : 2 * D, a, qsl], tmp[:])
                W.progress += 1

            for qt in range(NQT):
                for a in range(CT):
                    if a >= 1 and W.has(f"k{a}_{qt}"):
                        W.drain_until(f"k{a}_{qt}")
                    if a == 0 and qt >= 1 and W.has(f"k0_{qt}"):
                        W.drain_until(f"k0_{qt}")
                    emit_attn(qt, a)

            W.drain_all()

    nc.compile()
    return nc


def kernel(input, attention_mask, Wq, Wk, Wv, Wp, _profile=False):
    input = np.asarray(input, dtype=np.float32)
    attention_mask = np.asarray(attention_mask)
    Wq, Wk, Wv, Wp = (np.asarray(w, dtype=np.float32) for w in (Wq, Wk, Wv, Wp))

    mask_all = bool(attention_mask.all())
    nc = build_program(apply_key_mask=not mask_all)

    in_maps = []
    for core in range(8):
        b, g = core // 2, core % 2
        rows = slice(g * JC, (g + 1) * JC)
        m = {
            "xT": np.ascontiguousarray(input[b].T).astype(NP_BF16),
            "wqT": np.ascontiguousarray(Wq[rows].T).astype(NP_BF16),
            "wkT": np.ascontiguousarray(Wk[rows].T).astype(NP_BF16),
            "wvT": np.ascontiguousarray(Wv[rows].T).astype(NP_BF16),
            "wpT": np.ascontiguousarray(Wp[:, rows].T).astype(NP_BF16),
        }
        if not mask_all:
            km = attention_mask[b].astype(np.float32)  # [S]
            m["kmaskT"] = np.ascontiguousarray(km.reshape(NKT, P).T)
        in_maps.append(m)

    res = run_bass_kernel_spmd(
        nc, in_maps, core_ids=list(range(8)), trace=_profile
    )

    out = np.empty((B, S, E), dtype=np.float32)
    for b in range(B):
        acc = (res.results[2 * b]["outp"].astype(np.float32)
               + res.results[2 * b + 1]["outp"].astype(np.float32))
        out[b] = acc.T
    if _profile:
        return out, res
    return out
